# revision 4
# baseline (speedup 1.0000x reference)
"""Causal multi-head attention (B=4, T=2048, H=1024, 16 heads) on 8 trn2 cores.

Sharding: batch(4) x head-group(2).  Core c -> batch b=c//2, heads g=c%2
(8 heads each).  Each core computes its QKV projection slice, causal+padding
masked attention for its 8 heads, and a row-parallel slice of the output
projection.  The two partial outputs per batch row are summed on the host
(row-parallel unshard); b_out is folded into the device matmuls (b_out/16 per
ones-row, 16 ones-rows per batch across the two cores).

Device algorithm (per core, everything transposed so softmax reduces along
the PE contraction dim):
  xT [H, T] (host-pretransposed input row)
  QT/KT [512, T] = wqk^T-slices @ xT   (Q pre-scaled by 1/sqrt(hd) on host)
  V    [T, 520]  = xT^T @ wv (+bias), bf16, with a ones column per head
  per (head, q-tile 512, k-chunk 128):
     S^T[k, q] = KT_h[:, kchunk].T @ QT_h[:, qtile]   (f32r matmuls)
     (+ causal mask PSUM-preload on diagonal chunks, via identity matmul)
     P^T = exp(S^T + padbias[k])          (ScalarE, per-partition bias AP)
     o^T[65, q] += V_aug[kchunk, head].T @ P^T        (row 64 = softmax denom)
  o_scaled = o^T * (1/denom)  (recip on DVE, broadcast via gpsimd)
  y[t, j] = sum_h o_scaled_h[:, t].T @ wout_aug_h[:, j]
"""

import os
import sys

import numpy as np

sys.path.insert(0, "/opt/trn_rl_repo")

B, T, H = 4, 2048, 1024
NH, HD = 16, 64
NCORES = 8
HPC = 8          # heads per core
GD = HPC * HD    # head dims per core = 512
KC = T // 128    # 16 k-chunks
QT_TILES = T // 512  # 4 q-tiles
HC = H // 128    # 8 h-chunks (contraction for projections)

NEG = -1.0e9


def _build_nc():
    import concourse.bass as bass
    import concourse.tile as tile
    import concourse.mybir as mybir
    from concourse import bacc
    from contextlib import ExitStack

    f32 = mybir.dt.float32
    f32r = mybir.dt.float32r
    bf16 = mybir.dt.bfloat16
    EXP = mybir.ActivationFunctionType.Exp

    nc = bacc.Bacc("TRN2", target_bir_lowering=False, debug=False)

    xT_d = nc.dram_tensor("xT", [H, T], f32, kind="ExternalInput").ap()
    wqk_d = nc.dram_tensor("wqk", [H, 2 * GD], f32, kind="ExternalInput").ap()
    wv_d = nc.dram_tensor("wv", [H, GD], f32, kind="ExternalInput").ap()
    bqk_d = nc.dram_tensor("bqk", [1, 2 * GD], f32, kind="ExternalInput").ap()
    bv_d = nc.dram_tensor("bv", [1, GD], f32, kind="ExternalInput").ap()
    wout_d = nc.dram_tensor("wout", [HPC * 65, H], f32, kind="ExternalInput").ap()
    padb_d = nc.dram_tensor("padb", [128, KC], f32, kind="ExternalInput").ap()
    cmask_d = nc.dram_tensor("cmask", [128, 4 * 512], bf16, kind="ExternalInput").ap()
    ident_d = nc.dram_tensor("ident", [128, 128], bf16, kind="ExternalInput").ap()
    ones_d = nc.dram_tensor("ones", [1, 512], f32, kind="ExternalInput").ap()
    y_d = nc.dram_tensor("y", [T, H], f32, kind="ExternalOutput").ap()

    def r(ap):
        return ap.bitcast(f32r)

    with ExitStack() as ctx:
        tc = ctx.enter_context(tile.TileContext(nc))

        const = ctx.enter_context(tc.tile_pool(name="const", bufs=1))
        ident_sb = const.tile([128, 128], bf16, name="ident_sb")
        nc.sync.dma_start(ident_sb, ident_d)
        cmask_sb = const.tile([128, 4 * 512], bf16, name="cmask_sb")
        nc.sync.dma_start(cmask_sb, cmask_d)
        padb_sb = const.tile([128, KC], f32, name="padb_sb")
        nc.sync.dma_start(padb_sb, padb_d)
        bqk_sb = const.tile([1, 2 * GD], f32, name="bqk_sb")
        nc.sync.dma_start(r(bqk_sb), r(bqk_d))
        bv_sb = const.tile([1, GD], f32, name="bv_sb")
        nc.sync.dma_start(r(bv_sb), r(bv_d))
        ones_sb = const.tile([1, 512], f32, name="ones_sb")
        nc.sync.dma_start(r(ones_sb), r(ones_d))

        # Persistent activations
        acts = ctx.enter_context(tc.tile_pool(name="acts", bufs=1))
        qk_sb = [acts.tile([128, T], f32, name=f"qk{i}") for i in range(8)]
        v_sb = [acts.tile([128, HPC * 65], bf16, name=f"v{c}") for c in range(KC)]

        # ---------------- Phase 1: QKV projections ----------------
        with ExitStack() as p1:
            xt_pool = p1.enter_context(tc.tile_pool(name="xt", bufs=1))
            xt = [xt_pool.tile([128, T], f32, name=f"xt{i}") for i in range(HC)]
            for i in range(HC):
                nc.sync.dma_start(r(xt[i]), r(xT_d[i * 128:(i + 1) * 128, :]))

            wqk_pool = p1.enter_context(tc.tile_pool(name="wqkp", bufs=16))
            wv_pool = p1.enter_context(tc.tile_pool(name="wvp", bufs=8))
            ps1 = p1.enter_context(tc.tile_pool(name="ps1", bufs=4, space="PSUM"))

            # Q^T and K^T: out[col, t] tiles
            for ct in range(8):
                wts = []
                for hc in range(HC):
                    wt = wqk_pool.tile([128, 128], f32, tag="w", name=f"w{ct}_{hc}")
                    nc.sync.dma_start(
                        r(wt), r(wqk_d[hc * 128:(hc + 1) * 128, ct * 128:(ct + 1) * 128]))
                    wts.append(wt)
                for tt in range(4):
                    ps = ps1.tile([128, 512], f32, tag="ps", name=f"psqk{ct}_{tt}")
                    nc.tensor.matmul(
                        ps, r(bqk_sb[0:1, ct * 128:(ct + 1) * 128]), r(ones_sb),
                        start=True, stop=False)
                    for hc in range(HC):
                        nc.tensor.matmul(
                            ps, r(wts[hc]), r(xt[hc][:, tt * 512:(tt + 1) * 512]),
                            start=False, stop=(hc == HC - 1))
                    nc.vector.tensor_copy(r(qk_sb[ct][:, tt * 512:(tt + 1) * 512]), ps)

            # V: out[t, col] tiles, bf16, interleaved [8 heads x 65] with ones col
            wvts = []
            for hc in range(HC):
                wvt = wv_pool.tile([128, GD], f32, tag="wv", name=f"wv{hc}")
                nc.sync.dma_start(r(wvt), r(wv_d[hc * 128:(hc + 1) * 128, :]))
                wvts.append(wvt)
            for ts in range(KC):
                psv = ps1.tile([128, 512], f32, tag="psv", name=f"psv{ts}")
                nc.tensor.matmul(psv, r(ones_sb[0:1, 0:128]), r(bv_sb),
                                 start=True, stop=False)
                for hc in range(HC):
                    nc.tensor.matmul(
                        psv, r(xt[hc][:, ts * 128:(ts + 1) * 128]), r(wvts[hc]),
                        start=False, stop=(hc == HC - 1))
                # copy into interleaved bf16 layout: head h -> cols [65h, 65h+64)
                dst = v_sb[ts].rearrange("p (h c) -> p h c", h=HPC)[:, :, 0:64]
                src = psv.rearrange("p (h c) -> p h c", h=HPC)
                nc.vector.tensor_copy(dst, src)
                nc.vector.memset(
                    v_sb[ts].rearrange("p (h c) -> p h c", h=HPC)[:, :, 64:65], 1.0)

        # ---------------- Phase 2: attention + output projection ----------------
        with ExitStack() as p2:
            wout_pool = p2.enter_context(tc.tile_pool(name="woutp", bufs=1))
            wout_sb = [wout_pool.tile([65, H], f32, name=f"wo{h}") for h in range(HPC)]
            for h in range(HPC):
                nc.sync.dma_start(r(wout_sb[h]), r(wout_d[h * 65:(h + 1) * 65, :]))

            ppool = p2.enter_context(tc.tile_pool(name="pchunks", bufs=8))
            osc_pool = p2.enter_context(tc.tile_pool(name="osc", bufs=16))
            dpool = p2.enter_context(tc.tile_pool(name="dtiles", bufs=4))
            ypool = p2.enter_context(tc.tile_pool(name="ysb", bufs=4))
            ps_s = p2.enter_context(tc.tile_pool(name="ps_s", bufs=2, space="PSUM"))
            ps_o = p2.enter_context(tc.tile_pool(name="ps_o", bufs=2, space="PSUM"))
            ps_y = p2.enter_context(tc.tile_pool(name="ps_y", bufs=2, space="PSUM"))

            for qt in range(QT_TILES):
                q0 = qt * 512
                nk = 4 * (qt + 1)
                osc = []
                for h in range(HPC):
                    hq = qk_sb[h // 2][(h % 2) * 64:(h % 2) * 64 + 64, q0:q0 + 512]
                    opsum = ps_o.tile([65, 512], f32, tag="o", name=f"o{qt}_{h}")
                    for c in range(nk):
                        spsum = ps_s.tile([128, 512], f32, tag="s", name=f"s{qt}_{h}_{c}")
                        diag = c >= 4 * qt
                        if diag:
                            dd = c - 4 * qt
                            nc.tensor.matmul(
                                spsum, ident_sb, cmask_sb[:, dd * 512:(dd + 1) * 512],
                                start=True, stop=False)
                        hk = qk_sb[4 + h // 2][(h % 2) * 64:(h % 2) * 64 + 64,
                                               c * 128:(c + 1) * 128]
                        nc.tensor.matmul(spsum, r(hk), r(hq),
                                         start=not diag, stop=True)
                        pt = ppool.tile([128, 512], bf16, tag="p", name=f"p{qt}_{h}_{c}")
                        nc.scalar.activation(pt, spsum, EXP,
                                             bias=padb_sb[:, c:c + 1], scale=1.0)
                        nc.tensor.matmul(
                            opsum,
                            v_sb[c][:, h * 65:(h + 1) * 65].bitcast(bf16),
                            pt,
                            start=(c == 0), stop=(c == nk - 1))
                    # softmax denominator -> reciprocal -> broadcast -> scale
                    stage = dpool.tile([65, 512], f32, tag="dstage", name=f"st{qt}_{h}")
                    nc.vector.tensor_copy(stage[64:65, :], opsum[64:65, :])
                    dp0 = dpool.tile([1, 512], f32, tag="dp0", name=f"dp0_{qt}_{h}")
                    nc.sync.dma_start(dp0, stage[64:65, :])
                    rp0 = dpool.tile([1, 512], f32, tag="rp0", name=f"rp0_{qt}_{h}")
                    nc.vector.reciprocal_approx_fast(rp0, dp0)
                    rrep = dpool.tile([65, 512], f32, tag="rrep", name=f"rr{qt}_{h}")
                    nc.gpsimd.partition_broadcast(rrep, rp0)
                    o_sc = osc_pool.tile([65, 512], f32, tag="osc", name=f"osc{qt}_{h}")
                    nc.vector.tensor_mul(r(o_sc), rrep, opsum)
                    osc.append(o_sc)

                # output projection for this q-tile
                for j in range(2):
                    for ts in range(4):
                        ypsum = ps_y.tile([128, 512], f32, tag="y", name=f"y{qt}_{j}_{ts}")
                        for h in range(HPC):
                            nc.tensor.matmul(
                                ypsum,
                                r(osc[h][:, ts * 128:(ts + 1) * 128]),
                                r(wout_sb[h][:, j * 512:(j + 1) * 512]),
                                start=(h == 0), stop=(h == HPC - 1))
                        ysb = ypool.tile([128, 512], f32, tag="ysb", name=f"ys{qt}_{j}_{ts}")
                        nc.vector.tensor_copy(ysb, ypsum)
                        nc.sync.dma_start(
                            y_d[q0 + ts * 128:q0 + (ts + 1) * 128, j * 512:(j + 1) * 512],
                            ysb)

    nc.compile()
    return nc


_NC_CACHE = None


def _get_nc():
    global _NC_CACHE
    if _NC_CACHE is None:
        _NC_CACHE = _build_nc()
    return _NC_CACHE


def make_core_inputs(input, mask, w_qkv, b_qkv, w_out, b_out, core):
    """Host-side sharding/layout prep for one core."""
    b, g = core // 2, core % 2
    scale = 1.0 / np.sqrt(HD)

    xT = np.ascontiguousarray(input[b].T).astype(np.float32)          # [H, T]

    qcols = slice(g * GD, (g + 1) * GD)
    kcols = slice(H + g * GD, H + (g + 1) * GD)
    vcols = slice(2 * H + g * GD, 2 * H + (g + 1) * GD)
    wq = w_qkv[:, qcols] * scale
    wk = w_qkv[:, kcols]
    wqk = np.ascontiguousarray(np.concatenate([wq, wk], axis=1)).astype(np.float32)
    bqk = np.concatenate([b_qkv[qcols] * scale, b_qkv[kcols]])[None, :].astype(np.float32)
    wv = np.ascontiguousarray(w_qkv[:, vcols]).astype(np.float32)
    bv = b_qkv[vcols][None, :].astype(np.float32)

    # wout augmented: per head 64 rows of w_out + one b_out/16 row
    wout = np.empty((HPC * 65, H), dtype=np.float32)
    for h in range(HPC):
        rows = slice((g * HPC + h) * HD, (g * HPC + h + 1) * HD)
        wout[h * 65:h * 65 + 64] = w_out[rows]
        wout[h * 65 + 64] = b_out / 16.0

    padb = np.where(mask[b], np.float32(0.0), np.float32(NEG)).astype(np.float32)
    padb = np.ascontiguousarray(padb.reshape(KC, 128).T)              # [128, KC]

    # 4 causal diagonal mask patterns: delta = 128*dd; valid iff col >= row + delta
    import ml_dtypes
    cm = np.empty((128, 4 * 512), dtype=np.float32)
    rr = np.arange(128)[:, None]
    cc = np.arange(512)[None, :]
    for dd in range(4):
        cm[:, dd * 512:(dd + 1) * 512] = np.where(cc >= rr + 128 * dd, 0.0, NEG)
    cmask = cm.astype(ml_dtypes.bfloat16)
    ident = np.eye(128, dtype=np.float32).astype(ml_dtypes.bfloat16)
    ones = np.ones((1, 512), dtype=np.float32)

    return {
        "xT": xT, "wqk": wqk, "wv": wv, "bqk": bqk, "bv": bv,
        "wout": wout, "padb": padb, "cmask": cmask, "ident": ident,
        "ones": ones,
    }


def kernel(input, mask, w_qkv, b_qkv, w_out, b_out):
    from concourse.bass_utils import run_bass_kernel_spmd

    nc = _get_nc()
    in_maps = [
        make_core_inputs(input, mask, w_qkv, b_qkv, w_out, b_out, c)
        for c in range(NCORES)
    ]
    res = run_bass_kernel_spmd(nc, in_maps, list(range(NCORES)))
    parts = [res.results[c]["y"] for c in range(NCORES)]
    out = np.stack([parts[2 * b] + parts[2 * b + 1] for b in range(B)])
    return out.astype(np.float32)


if __name__ == "__main__":
    # smoke build
    nc = _build_nc()
    print("build ok:", len(nc.m.functions[0].instructions) if hasattr(nc.m.functions[0], "instructions") else "?")


# revision 8
# speedup vs baseline: 1.4391x; 1.4391x over previous
"""Causal multi-head attention (B=4, T=2048, H=1024, 16 heads) on 8 trn2 cores.

Sharding: batch(4) x head-group(2).  Core c -> batch b=c//2, heads g=c%2
(8 heads each).  Each core computes its QKV projection slice, causal+padding
masked attention for its 8 heads, and a row-parallel slice of the output
projection.  The two partial outputs per batch row are summed on the host
(row-parallel unshard); b_out is folded into the device matmuls (b_out/16 per
ones-row, 16 ones-rows per batch across the two cores).

Device algorithm (per core, everything transposed so softmax reduces along
the PE contraction dim):
  xT [H, T] (host-pretransposed input row)
  QT/KT [512, T] = wqk^T-slices @ xT   (Q pre-scaled by 1/sqrt(hd) on host)
  V    [T, 520]  = xT^T @ wv (+bias), bf16, with a ones column per head
  per (head, q-tile 512, k-chunk 128):
     S^T[k, q] = KT_h[:, kchunk].T @ QT_h[:, qtile]   (f32r matmuls)
     (+ causal mask PSUM-preload on diagonal chunks, via identity matmul)
     P^T = exp(S^T + padbias[k])          (ScalarE, per-partition bias AP)
     o^T[65, q] += V_aug[kchunk, head].T @ P^T        (row 64 = softmax denom)
  o_scaled = o^T * (1/denom)  (recip on DVE, broadcast via gpsimd)
  y[t, j] = sum_h o_scaled_h[:, t].T @ wout_aug_h[:, j]
"""

import os
import sys

import numpy as np

sys.path.insert(0, "/opt/trn_rl_repo")

B, T, H = 4, 2048, 1024
NH, HD = 16, 64
NCORES = 8
HPC = 8          # heads per core
GD = HPC * HD    # head dims per core = 512
KC = T // 128    # 16 k-chunks
QT_TILES = T // 512  # 4 q-tiles
HC = H // 128    # 8 h-chunks (contraction for projections)

NEG = -1.0e9


def _build_nc():
    import concourse.bass as bass
    import concourse.tile as tile
    import concourse.mybir as mybir
    from concourse import bacc
    from contextlib import ExitStack

    f32 = mybir.dt.float32
    f32r = mybir.dt.float32r
    bf16 = mybir.dt.bfloat16
    EXP = mybir.ActivationFunctionType.Exp

    nc = bacc.Bacc("TRN2", target_bir_lowering=False, debug=False)

    xT_d = nc.dram_tensor("xT", [H, T], f32, kind="ExternalInput").ap()
    wqk_d = nc.dram_tensor("wqk", [H, 2 * GD], f32, kind="ExternalInput").ap()
    wv_d = nc.dram_tensor("wv", [H, GD], f32, kind="ExternalInput").ap()
    bqk_d = nc.dram_tensor("bqk", [1, 2 * GD], f32, kind="ExternalInput").ap()
    bv_d = nc.dram_tensor("bv", [1, GD], f32, kind="ExternalInput").ap()
    wout_d = nc.dram_tensor("wout", [HPC * 65, H], f32, kind="ExternalInput").ap()
    padb_d = nc.dram_tensor("padb", [128, KC], f32, kind="ExternalInput").ap()
    cmask_d = nc.dram_tensor("cmask", [128, 4 * 512], bf16, kind="ExternalInput").ap()
    ident_d = nc.dram_tensor("ident", [128, 128], bf16, kind="ExternalInput").ap()
    ones_d = nc.dram_tensor("ones", [1, 512], f32, kind="ExternalInput").ap()
    y_d = nc.dram_tensor("y", [T, H], f32, kind="ExternalOutput").ap()

    def r(ap):
        return ap.bitcast(f32r)

    with ExitStack() as ctx:
        tc = ctx.enter_context(tile.TileContext(nc))

        const = ctx.enter_context(tc.tile_pool(name="const", bufs=1))
        ident_sb = const.tile([128, 128], bf16, name="ident_sb")
        nc.sync.dma_start(ident_sb, ident_d)
        cmask_sb = const.tile([128, 4 * 512], bf16, name="cmask_sb")
        nc.sync.dma_start(cmask_sb, cmask_d)
        padb_sb = const.tile([128, KC], f32, name="padb_sb")
        nc.sync.dma_start(padb_sb, padb_d)
        bqk_sb = const.tile([1, 2 * GD], f32, name="bqk_sb")
        nc.sync.dma_start(r(bqk_sb), r(bqk_d))
        bv_sb = const.tile([1, GD], f32, name="bv_sb")
        nc.sync.dma_start(r(bv_sb), r(bv_d))
        ones_sb = const.tile([1, 512], f32, name="ones_sb")
        nc.sync.dma_start(r(ones_sb), r(ones_d))

        # Persistent activations
        acts = ctx.enter_context(tc.tile_pool(name="acts", bufs=1))
        qk_sb = [acts.tile([128, T], f32, name=f"qk{i}") for i in range(8)]
        v_sb = [acts.tile([128, HPC * 65], bf16, name=f"v{c}") for c in range(KC)]

        # ---------------- Phase 1: QKV projections ----------------
        with ExitStack() as p1:
            xt_pool = p1.enter_context(tc.tile_pool(name="xt", bufs=1))
            xt = [xt_pool.tile([128, T], f32, name=f"xt{i}") for i in range(HC)]
            for i in range(HC):
                nc.sync.dma_start(r(xt[i]), r(xT_d[i * 128:(i + 1) * 128, :]))

            wqk_pool = p1.enter_context(tc.tile_pool(name="wqkp", bufs=16))
            wv_pool = p1.enter_context(tc.tile_pool(name="wvp", bufs=8))
            ps1 = p1.enter_context(tc.tile_pool(name="ps1", bufs=4, space="PSUM"))

            # Q^T and K^T: out[col, t] tiles
            for ct in range(8):
                wts = []
                for hc in range(HC):
                    wt = wqk_pool.tile([128, 128], f32, tag="w", name=f"w{ct}_{hc}")
                    nc.sync.dma_start(
                        r(wt), r(wqk_d[hc * 128:(hc + 1) * 128, ct * 128:(ct + 1) * 128]))
                    wts.append(wt)
                for tt in range(4):
                    ps = ps1.tile([128, 512], f32, tag="ps", name=f"psqk{ct}_{tt}")
                    nc.tensor.matmul(
                        ps, r(bqk_sb[0:1, ct * 128:(ct + 1) * 128]), r(ones_sb),
                        start=True, stop=False)
                    for hc in range(HC):
                        nc.tensor.matmul(
                            ps, r(wts[hc]), r(xt[hc][:, tt * 512:(tt + 1) * 512]),
                            start=False, stop=(hc == HC - 1))
                    nc.vector.tensor_copy(r(qk_sb[ct][:, tt * 512:(tt + 1) * 512]), ps)

            # V: out[t, col] tiles, bf16, interleaved [8 heads x 65] with ones col
            wvts = []
            for hc in range(HC):
                wvt = wv_pool.tile([128, GD], f32, tag="wv", name=f"wv{hc}")
                nc.sync.dma_start(r(wvt), r(wv_d[hc * 128:(hc + 1) * 128, :]))
                wvts.append(wvt)
            for ts in range(KC):
                psv = ps1.tile([128, 512], f32, tag="psv", name=f"psv{ts}")
                nc.tensor.matmul(psv, r(ones_sb[0:1, 0:128]), r(bv_sb),
                                 start=True, stop=False)
                for hc in range(HC):
                    nc.tensor.matmul(
                        psv, r(xt[hc][:, ts * 128:(ts + 1) * 128]), r(wvts[hc]),
                        start=False, stop=(hc == HC - 1))
                # copy into interleaved bf16 layout: head h -> cols [65h, 65h+64)
                dst = v_sb[ts].rearrange("p (h c) -> p h c", h=HPC)[:, :, 0:64]
                src = psv.rearrange("p (h c) -> p h c", h=HPC)
                nc.vector.tensor_copy(dst, src)
                nc.vector.memset(
                    v_sb[ts].rearrange("p (h c) -> p h c", h=HPC)[:, :, 64:65], 1.0)

        # ---------------- Phase 2: attention + output projection ----------------
        with ExitStack() as p2:
            wout_pool = p2.enter_context(tc.tile_pool(name="woutp", bufs=1))
            wout_sb = [wout_pool.tile([65, H], f32, name=f"wo{h}") for h in range(HPC)]
            for h in range(HPC):
                nc.sync.dma_start(r(wout_sb[h]), r(wout_d[h * 65:(h + 1) * 65, :]))

            ppool = p2.enter_context(tc.tile_pool(name="pchunks", bufs=20))
            osc_pool = p2.enter_context(tc.tile_pool(name="osc", bufs=16))
            dpool = p2.enter_context(tc.tile_pool(name="dtiles", bufs=2))
            ypool = p2.enter_context(tc.tile_pool(name="ysb", bufs=4))
            ps_s = p2.enter_context(tc.tile_pool(name="ps_s", bufs=4, space="PSUM"))
            ps_o = p2.enter_context(tc.tile_pool(name="ps_o", bufs=2, space="PSUM"))
            ps_y = p2.enter_context(tc.tile_pool(name="ps_y", bufs=2, space="PSUM"))

            for qt in range(QT_TILES):
                q0 = qt * 512
                nk = 4 * (qt + 1)
                osc = []
                for h in range(HPC):
                    hq = qk_sb[h // 2][(h % 2) * 64:(h % 2) * 64 + 64, q0:q0 + 512]
                    opsum = ps_o.tile([65, 512], f32, tag="o", name=f"o{qt}_{h}")
                    # All S matmuls first (exp trails on ScalarE), then all PV
                    # matmuls: keeps the PE stream dense so HAM stays warm.
                    pts = []
                    for c in range(nk):
                        spsum = ps_s.tile([128, 512], f32, tag="s", name=f"s{qt}_{h}_{c}")
                        diag = c >= 4 * qt
                        if diag:
                            dd = c - 4 * qt
                            nc.tensor.matmul(
                                spsum, ident_sb, cmask_sb[:, dd * 512:(dd + 1) * 512],
                                start=True, stop=False)
                        hk = qk_sb[4 + h // 2][(h % 2) * 64:(h % 2) * 64 + 64,
                                               c * 128:(c + 1) * 128]
                        nc.tensor.matmul(spsum, r(hk), r(hq),
                                         start=not diag, stop=True)
                        pt = ppool.tile([128, 512], bf16, tag="p", name=f"p{qt}_{h}_{c}")
                        nc.scalar.activation(pt, spsum, EXP,
                                             bias=padb_sb[:, c:c + 1], scale=1.0)
                        pts.append(pt)
                    for c in range(nk):
                        nc.tensor.matmul(
                            opsum,
                            v_sb[c][:, h * 65:(h + 1) * 65].bitcast(bf16),
                            pts[c],
                            start=(c == 0), stop=(c == nk - 1))
                    # softmax denominator -> reciprocal -> broadcast -> scale
                    stage = dpool.tile([65, 512], f32, tag="dstage", name=f"st{qt}_{h}")
                    nc.vector.tensor_copy(stage[64:65, :], opsum[64:65, :])
                    dp0 = dpool.tile([1, 512], f32, tag="dp0", name=f"dp0_{qt}_{h}")
                    nc.sync.dma_start(dp0, stage[64:65, :])
                    rp0 = dpool.tile([1, 512], f32, tag="rp0", name=f"rp0_{qt}_{h}")
                    nc.vector.reciprocal_approx_fast(rp0, dp0)
                    rrep = dpool.tile([65, 512], f32, tag="rrep", name=f"rr{qt}_{h}")
                    nc.gpsimd.partition_broadcast(rrep, rp0)
                    o_sc = osc_pool.tile([65, 512], f32, tag="osc", name=f"osc{qt}_{h}")
                    nc.vector.tensor_mul(r(o_sc), rrep, opsum)
                    osc.append(o_sc)

                # output projection for this q-tile
                for j in range(2):
                    for ts in range(4):
                        ypsum = ps_y.tile([128, 512], f32, tag="y", name=f"y{qt}_{j}_{ts}")
                        for h in range(HPC):
                            nc.tensor.matmul(
                                ypsum,
                                r(osc[h][:, ts * 128:(ts + 1) * 128]),
                                r(wout_sb[h][:, j * 512:(j + 1) * 512]),
                                start=(h == 0), stop=(h == HPC - 1))
                        ysb = ypool.tile([128, 512], f32, tag="ysb", name=f"ys{qt}_{j}_{ts}")
                        nc.vector.tensor_copy(ysb, ypsum)
                        nc.sync.dma_start(
                            y_d[q0 + ts * 128:q0 + (ts + 1) * 128, j * 512:(j + 1) * 512],
                            ysb)

    nc.compile()
    return nc


_NC_CACHE = None


def _get_nc():
    global _NC_CACHE
    if _NC_CACHE is None:
        _NC_CACHE = _build_nc()
    return _NC_CACHE


def make_core_inputs(input, mask, w_qkv, b_qkv, w_out, b_out, core):
    """Host-side sharding/layout prep for one core."""
    b, g = core // 2, core % 2
    scale = 1.0 / np.sqrt(HD)

    xT = np.ascontiguousarray(input[b].T).astype(np.float32)          # [H, T]

    qcols = slice(g * GD, (g + 1) * GD)
    kcols = slice(H + g * GD, H + (g + 1) * GD)
    vcols = slice(2 * H + g * GD, 2 * H + (g + 1) * GD)
    wq = w_qkv[:, qcols] * scale
    wk = w_qkv[:, kcols]
    wqk = np.ascontiguousarray(np.concatenate([wq, wk], axis=1)).astype(np.float32)
    bqk = np.concatenate([b_qkv[qcols] * scale, b_qkv[kcols]])[None, :].astype(np.float32)
    wv = np.ascontiguousarray(w_qkv[:, vcols]).astype(np.float32)
    bv = b_qkv[vcols][None, :].astype(np.float32)

    # wout augmented: per head 64 rows of w_out + one b_out/16 row
    wout = np.empty((HPC * 65, H), dtype=np.float32)
    for h in range(HPC):
        rows = slice((g * HPC + h) * HD, (g * HPC + h + 1) * HD)
        wout[h * 65:h * 65 + 64] = w_out[rows]
        wout[h * 65 + 64] = b_out / 16.0

    padb = np.where(mask[b], np.float32(0.0), np.float32(NEG)).astype(np.float32)
    padb = np.ascontiguousarray(padb.reshape(KC, 128).T)              # [128, KC]

    # 4 causal diagonal mask patterns: delta = 128*dd; valid iff col >= row + delta
    import ml_dtypes
    cm = np.empty((128, 4 * 512), dtype=np.float32)
    rr = np.arange(128)[:, None]
    cc = np.arange(512)[None, :]
    for dd in range(4):
        cm[:, dd * 512:(dd + 1) * 512] = np.where(cc >= rr + 128 * dd, 0.0, NEG)
    cmask = cm.astype(ml_dtypes.bfloat16)
    ident = np.eye(128, dtype=np.float32).astype(ml_dtypes.bfloat16)
    ones = np.ones((1, 512), dtype=np.float32)

    return {
        "xT": xT, "wqk": wqk, "wv": wv, "bqk": bqk, "bv": bv,
        "wout": wout, "padb": padb, "cmask": cmask, "ident": ident,
        "ones": ones,
    }


def kernel(input, mask, w_qkv, b_qkv, w_out, b_out):
    from concourse.bass_utils import run_bass_kernel_spmd

    nc = _get_nc()
    in_maps = [
        make_core_inputs(input, mask, w_qkv, b_qkv, w_out, b_out, c)
        for c in range(NCORES)
    ]
    res = run_bass_kernel_spmd(nc, in_maps, list(range(NCORES)))
    parts = [res.results[c]["y"] for c in range(NCORES)]
    out = np.stack([parts[2 * b] + parts[2 * b + 1] for b in range(B)])
    return out.astype(np.float32)


if __name__ == "__main__":
    # smoke build
    nc = _build_nc()
    print("build ok:", len(nc.m.functions[0].instructions) if hasattr(nc.m.functions[0], "instructions") else "?")
